# revision 43
# baseline (speedup 1.0000x reference)
"""GroupPearson Trainium2 kernel, v13: PE 2x2-Gram diagonal segment reduction
with size-sorted variable-depth windows.

Sharding: host sorts groups by size (ascending), assigns rank r to
(window slot j = (r//64)//8, core = (r//64)%8, column m = r%64), and
pads each window slot to T_j = ceil(max_size_in_slot/128) slabs of 128
rows.  It ships one fp8(e4m3) tensor per core: slab (j, t) is a
[128, 129] block with columns [x of 64 groups | y of 64 groups | ones].

Device, per window slot j and slab t - ONE matmul with
  lhsT = slab[:, 0:128] ([x64 | y64]), rhs = slab[:, 0:129]:
  psum[m,      m     ] += sum_k x*x   (sxx)
  psum[m,      64 + m] += sum_k x*y   (sxy)
  psum[64 + m, 64 + m] += sum_k y*y   (syy)
  psum[m,      128   ] += sum_k x     (sx)
  psum[64 + m, 128   ] += sum_k y     (sy)
accumulated over t in fp32 PSUM (only the first matmul into a bank may
carry start=True - start clears has_written for the whole bank).
Window slots are processed in pairs with interleaved slabs so each
matmul hides the partner's LDWEIGHTS.  DVE extracts the diagonals with
an identity-mask scalar_tensor_tensor reduce and copies the
ones-column; host finishes the correlation in float64.
"""

import numpy as np
import ml_dtypes

P = 128
H = 64                       # groups per window slot
C = 129                      # slab columns: x(64) | y(64) | ones
G = 4096
N_CORES = 8
GPC = G // N_CORES           # 512 groups per core
NW = GPC // H                # 8 window slots per core
NO = 3                       # output cols per window slot
USE_FP8 = True


def build_nc(t_slots, n_devices=N_CORES):
    from concourse import mybir, tile, bacc
    from contextlib import ExitStack

    dt = mybir.dt
    OP = mybir.AluOpType
    ddt = dt.float8e4 if USE_FP8 else dt.bfloat16

    F = sum(t_slots) * C
    offs = np.concatenate([[0], np.cumsum(np.array(t_slots) * C)])

    nc = bacc.Bacc("TRN2", target_bir_lowering=False, debug=False,
                   num_devices=n_devices)
    v_d = nc.dram_tensor("v", [P, F], ddt, kind="ExternalInput").ap()
    id_d = nc.dram_tensor("ident", [P, H], dt.bfloat16,
                          kind="ExternalInput").ap()
    o_d = nc.dram_tensor("o", [P, NW * NO], dt.float32,
                         kind="ExternalOutput").ap()

    with tile.TileContext(nc) as tc, ExitStack() as ctx:
        const_pool = ctx.enter_context(tc.tile_pool(name="const", bufs=1))
        out_pool = ctx.enter_context(tc.tile_pool(name="out", bufs=1))
        io_pool = ctx.enter_context(tc.tile_pool(name="io", bufs=1))
        scr_pool = ctx.enter_context(tc.tile_pool(name="scr", bufs=2))
        ps_pool = ctx.enter_context(tc.psum_pool(name="ps", bufs=3))
        warm_pool = ctx.enter_context(tc.psum_pool(name="warm", bufs=1))

        # input DMAs first: one fat dma per slot-pair, alternating the two
        # HWDGE queues, in PROCESSING order (pair k dispatched k-th)
        ws = {}
        for k in range(NW // 2):
            fa, fb = int(offs[2 * k]), int(offs[2 * k + 2])
            pair = io_pool.tile([P, fb - fa], ddt, tag=f"pair{k}")
            mid = int(offs[2 * k + 1]) - fa
            eng = nc.sync if k % 2 == 0 else nc.scalar
            eng.dma_start(out=pair[:, :], in_=v_d[:, fa:fb])
            ws[2 * k] = pair[:, 0:mid]
            ws[2 * k + 1] = pair[:, mid:fb - fa]

        ident = const_pool.tile([P, H], dt.bfloat16)
        nc.sync.dma_start(out=ident[:, :], in_=id_d)

        # HAM warm-up: ~3us of dummy matmuls while the first pair's DMA is
        # in flight, so the PE clock is at 2.4GHz when real matmuls start
        # (cold K=4/8 would run the first ~3.4us of the stream at 1.2GHz)
        wdum = const_pool.tile([P, 512], ddt)
        nc.vector.memset(wdum[:, :], 0)
        wps = warm_pool.tile([P, 512], dt.float32)
        for _ in range(10):
            nc.tensor.matmul(wps[:, :], lhsT=wdum[:, 0:P], rhs=wdum[:, :],
                             start=True, stop=True, skip_group_check=True)

        outs = out_pool.tile([P, NW * NO], dt.float32)

        def extract(ps, w):
            ob = w * NO
            # diagonals: sxx (rows 0:64), syy (rows 64:128), sxy (0:64)
            scr = scr_pool.tile([P, H], dt.float32, tag="scr")
            nc.vector.scalar_tensor_tensor(
                scr[0:H, :], ps[0:H, 0:H], 1.0, ident[0:H, :], OP.mult,
                OP.mult, accum_out=outs[0:H, ob:ob + 1])
            nc.vector.scalar_tensor_tensor(
                scr[H:P, :], ps[H:P, H:2 * H], 1.0, ident[H:P, :], OP.mult,
                OP.mult, accum_out=outs[H:P, ob:ob + 1])
            scr2 = scr_pool.tile([P, H], dt.float32, tag="scr2")
            nc.vector.scalar_tensor_tensor(
                scr2[0:H, :], ps[0:H, H:2 * H], 1.0, ident[0:H, :], OP.mult,
                OP.mult, accum_out=outs[0:H, ob + 1:ob + 2])
            # ones column: sx (rows 0:64), sy (rows 64:128) - on the idle
            # ACT engine, parallel to the DVE diagonal extracts
            nc.scalar.copy(outs[:, ob + 2:ob + 3], ps[:, P:P + 1])

        # slot pairs with interleaved slabs: MM(wa,t), MM(wb,t), ... so
        # each matmul hides the other stream's LDWEIGHTS
        for k in range(NW // 2):
            wa, wb = 2 * k, 2 * k + 1
            ta, tb = t_slots[wa], t_slots[wb]
            sa = ws[wa].rearrange("p (t c) -> p t c", c=C)
            sb = ws[wb].rearrange("p (t c) -> p t c", c=C)
            pa = ps_pool.tile([P, C], dt.float32, tag="psa")
            pb = ps_pool.tile([P, C], dt.float32, tag="psb")
            for t in range(max(ta, tb)):
                if t < ta:
                    nc.tensor.matmul(pa[:, :], lhsT=sa[:, t, 0:P],
                                     rhs=sa[:, t, :], start=(t == 0),
                                     stop=(t == ta - 1),
                                     skip_group_check=True)
                if t < tb:
                    nc.tensor.matmul(pb[:, :], lhsT=sb[:, t, 0:P],
                                     rhs=sb[:, t, :], start=(t == 0),
                                     stop=(t == tb - 1),
                                     skip_group_check=True)
            extract(pa, wa)
            extract(pb, wb)

        nc.sync.dma_start(out=o_d[:, :], in_=outs[:, :])

    nc.compile()
    return nc


def host_layout(pred, exp, group):
    """Size-sorted Gram-slab layout: per-core [P, F] fp8 + rank order."""
    from concourse import mybir
    ddt = mybir.dt.np(mybir.dt.float8e4) if USE_FP8 else ml_dtypes.bfloat16

    x = np.asarray(exp, dtype=np.float32)
    y = np.asarray(pred, dtype=np.float32)
    g = np.asarray(group).astype(np.int32)
    n = g.shape[0]

    sizes = np.bincount(g, minlength=G)
    # rank groups by size asc; rank r -> slot j = (r//64)//8, core (r//64)%8
    # (ascending: first processed pair is smallest -> shortest fill stall)
    grank = np.argsort(sizes, kind="stable")       # rank -> group id
    # per-slot depth: slot j spans ranks [512j, 512(j+1)); max is the last
    t_slots = tuple(
        max(int(np.ceil(int(sizes[grank[512 * j + 511]]) / P)), 1)
        for j in range(NW))
    offs = np.concatenate([[0], np.cumsum(np.array(t_slots))])  # slab offsets

    # destination of element i (of sorted-by-group stream):
    # group g at rank r: core = (r//64)%8, slot j, col m = r%64
    rank_of = np.empty(G, dtype=np.int64)
    rank_of[grank] = np.arange(G)
    order = np.argsort(g, kind="stable")
    gs = g[order].astype(np.int64)
    starts = np.zeros(G, dtype=np.int64)
    starts[1:] = np.cumsum(sizes)[:-1]
    pos = np.arange(n, dtype=np.int64) - starts[gs]   # position within group

    r = rank_of[gs]
    core = (r // H) % N_CORES
    slot = (r // H) // N_CORES
    col = r % H
    t = pos // P
    k = pos % P
    F = int(offs[-1]) * C
    # dst within [core][k][F]: (offs[slot] + t)*C + col_within_slab
    v = np.zeros((N_CORES, P, F), dtype=ddt)
    flat_f = (offs[slot] + t) * C
    for si, vv in enumerate((x, y)):
        v[core, k, flat_f + col + si * H] = vv.astype(ddt)[order]
    # ones columns
    for j in range(NW):
        for t_ in range(t_slots[j]):
            v[:, :, (int(offs[j]) + t_) * C + 2 * H] = ddt(1.0)
    return v, sizes.astype(np.float64), t_slots, grank


def _finish_host(S):
    n, sx, sy, sxy, sxx, syy = S
    n_safe = np.where(n > 0, n, 1.0)
    mx = sx / n_safe
    my = sy / n_safe
    cov = sxy / n_safe - mx * my
    var_x = sxx / n_safe - mx * mx
    var_y = syy / n_safe - my * my
    denom = np.sqrt(np.maximum(var_x * var_y, 0.0))
    corr = np.where(denom > 0, cov / np.where(denom > 0, denom, 1.0), 0.0)
    corr_pearson = np.sum(corr * n) / np.sum(n)
    return np.float32(-corr_pearson)


_NC_CACHE = {}


def _get_nc(t_slots):
    if t_slots not in _NC_CACHE:
        _NC_CACHE[t_slots] = build_nc(t_slots)
    return _NC_CACHE[t_slots]


def _install_ntff_hook():
    """bass_utils imports antenv.axon_hooks when tracing; this image lacks
    that submodule.  Provide it, wired to the axon ctypes NTFF hook, so a
    tracing harness does not crash.  Harmless when tracing is off."""
    import sys
    import types

    if "antenv.axon_hooks" in sys.modules:
        return
    try:
        import antenv

        mod = types.ModuleType("antenv.axon_hooks")
        hook = [None]
        mod.set_axon_ntff_profile_hook = lambda h: hook.__setitem__(0, h)
        mod.get_axon_ntff_profile_hook = lambda: hook[0]
        sys.modules["antenv.axon_hooks"] = mod
        antenv.axon_hooks = mod
        from trn_agent_boot.trn_boot import _ntff_profile_via_ctypes

        mod.set_axon_ntff_profile_hook(
            _ntff_profile_via_ctypes("/opt/axon/libaxon_pjrt.so"))
    except Exception:
        pass


def kernel(pred, exp, group, num_groups, _trace=False):
    _install_ntff_hook()
    from concourse.bass_utils import run_bass_kernel_spmd

    pred = np.asarray(pred)
    exp = np.asarray(exp)
    group = np.asarray(group)

    v, sizes, t_slots, grank = host_layout(pred, exp, group)
    nc = _get_nc(t_slots)

    idh = np.eye(H, dtype=ml_dtypes.bfloat16)
    ident = np.concatenate([idh, idh], axis=0)          # [128, 64]
    in_maps = [{"v": v[i], "ident": ident} for i in range(N_CORES)]

    res = run_bass_kernel_spmd(nc, in_maps, list(range(N_CORES)),
                               trace=_trace)

    # stats by rank: rank r = 64*(8*slot + core) + m
    Sr = np.zeros((5, G), dtype=np.float64)
    for i in range(N_CORES):
        o = res.results[i]["o"].astype(np.float64).reshape(P, NW, NO)
        r0 = H * i                                    # slot j block offset
        for j in range(NW):
            sl = slice(H * (N_CORES * j + i), H * (N_CORES * j + i) + H)
            Sr[3, sl] = o[0:H, j, 0]                  # sxx
            Sr[4, sl] = o[H:P, j, 0]                  # syy
            Sr[2, sl] = o[0:H, j, 1]                  # sxy
            Sr[0, sl] = o[0:H, j, 2]                  # sx
            Sr[1, sl] = o[H:P, j, 2]                  # sy
    S = np.zeros((6, G), dtype=np.float64)
    S[0] = sizes
    S[1][grank] = Sr[0]
    S[2][grank] = Sr[1]
    S[3][grank] = Sr[2]
    S[4][grank] = Sr[3]
    S[5][grank] = Sr[4]
    out = _finish_host(S)
    if _trace:
        return out, res
    return out


# revision 44
# speedup vs baseline: 1.0195x; 1.0195x over previous
"""GroupPearson Trainium2 kernel, v13: PE 2x2-Gram diagonal segment reduction
with size-sorted variable-depth windows.

Sharding: host sorts groups by size (ascending), assigns rank r to
(window slot j = (r//64)//8, core = (r//64)%8, column m = r%64), and
pads each window slot to T_j = ceil(max_size_in_slot/128) slabs of 128
rows.  It ships one fp8(e4m3) tensor per core: slab (j, t) is a
[128, 129] block with columns [x of 64 groups | y of 64 groups | ones].

Device, per window slot j and slab t - ONE matmul with
  lhsT = slab[:, 0:128] ([x64 | y64]), rhs = slab[:, 0:129]:
  psum[m,      m     ] += sum_k x*x   (sxx)
  psum[m,      64 + m] += sum_k x*y   (sxy)
  psum[64 + m, 64 + m] += sum_k y*y   (syy)
  psum[m,      128   ] += sum_k x     (sx)
  psum[64 + m, 128   ] += sum_k y     (sy)
accumulated over t in fp32 PSUM (only the first matmul into a bank may
carry start=True - start clears has_written for the whole bank).
Window slots are processed in pairs with interleaved slabs so each
matmul hides the partner's LDWEIGHTS.  DVE extracts the diagonals with
an identity-mask scalar_tensor_tensor reduce and copies the
ones-column; host finishes the correlation in float64.
"""

import numpy as np
import ml_dtypes

P = 128
H = 64                       # groups per window slot
C = 129                      # slab columns: x(64) | y(64) | ones
G = 4096
N_CORES = 8
GPC = G // N_CORES           # 512 groups per core
NW = GPC // H                # 8 window slots per core
NO = 3                       # output cols per window slot
USE_FP8 = True


def build_nc(t_slots, n_devices=N_CORES):
    from concourse import mybir, tile, bacc
    from contextlib import ExitStack

    dt = mybir.dt
    OP = mybir.AluOpType
    ddt = dt.float8e4 if USE_FP8 else dt.bfloat16

    F = sum(t_slots) * C
    offs = np.concatenate([[0], np.cumsum(np.array(t_slots) * C)])

    nc = bacc.Bacc("TRN2", target_bir_lowering=False, debug=False,
                   num_devices=n_devices)
    v_d = nc.dram_tensor("v", [P, F], ddt, kind="ExternalInput").ap()
    id_d = nc.dram_tensor("ident", [P, H], dt.bfloat16,
                          kind="ExternalInput").ap()
    o_d = nc.dram_tensor("o", [P, NW * NO], dt.float32,
                         kind="ExternalOutput").ap()

    with tile.TileContext(nc) as tc, ExitStack() as ctx:
        const_pool = ctx.enter_context(tc.tile_pool(name="const", bufs=1))
        out_pool = ctx.enter_context(tc.tile_pool(name="out", bufs=1))
        io_pool = ctx.enter_context(tc.tile_pool(name="io", bufs=1))
        scr_pool = ctx.enter_context(tc.tile_pool(name="scr", bufs=2))
        ps_pool = ctx.enter_context(tc.psum_pool(name="ps", bufs=3))
        warm_pool = ctx.enter_context(tc.psum_pool(name="warm", bufs=1))

        # input DMAs first: one fat dma per slot-pair, alternating the two
        # HWDGE queues, in PROCESSING order (pair k dispatched k-th)
        ws = {}
        for k in range(NW // 2):
            fa, fb = int(offs[2 * k]), int(offs[2 * k + 2])
            pair = io_pool.tile([P, fb - fa], ddt, tag=f"pair{k}")
            mid = int(offs[2 * k + 1]) - fa
            eng = nc.sync if k % 2 == 0 else nc.scalar
            eng.dma_start(out=pair[:, :], in_=v_d[:, fa:fb])
            ws[2 * k] = pair[:, 0:mid]
            ws[2 * k + 1] = pair[:, mid:fb - fa]

        ident = const_pool.tile([P, H], dt.bfloat16)
        nc.sync.dma_start(out=ident[:, :], in_=id_d)

        # HAM warm-up: ~3us of dummy matmuls while the first pair's DMA is
        # in flight, so the PE clock is at 2.4GHz when real matmuls start
        # (cold K=4/8 would run the first ~3.4us of the stream at 1.2GHz)
        wdum = const_pool.tile([P, 512], ddt)
        nc.vector.memset(wdum[:, :], 0)
        wps = warm_pool.tile([P, 512], dt.float32)
        for _ in range(10):
            nc.tensor.matmul(wps[:, :], lhsT=wdum[:, 0:P], rhs=wdum[:, :],
                             start=True, stop=True, skip_group_check=True)

        outs = out_pool.tile([P, NW * NO], dt.float32)

        def extract(ps, w):
            ob = w * NO
            # diagonals: sxx (rows 0:64), syy (rows 64:128), sxy (0:64)
            scr = scr_pool.tile([P, H], dt.float32, tag="scr")
            nc.vector.scalar_tensor_tensor(
                scr[0:H, :], ps[0:H, 0:H], 1.0, ident[0:H, :], OP.mult,
                OP.mult, accum_out=outs[0:H, ob:ob + 1])
            nc.vector.scalar_tensor_tensor(
                scr[H:P, :], ps[H:P, H:2 * H], 1.0, ident[H:P, :], OP.mult,
                OP.mult, accum_out=outs[H:P, ob:ob + 1])
            scr2 = scr_pool.tile([P, H], dt.float32, tag="scr2")
            nc.vector.scalar_tensor_tensor(
                scr2[0:H, :], ps[0:H, H:2 * H], 1.0, ident[0:H, :], OP.mult,
                OP.mult, accum_out=outs[0:H, ob + 1:ob + 2])
            # ones column: sx (rows 0:64), sy (rows 64:128)
            nc.vector.tensor_copy(outs[:, ob + 2:ob + 3], ps[:, P:P + 1])

        # slot pairs with interleaved slabs: MM(wa,t), MM(wb,t), ... so
        # each matmul hides the other stream's LDWEIGHTS
        for k in range(NW // 2):
            wa, wb = 2 * k, 2 * k + 1
            ta, tb = t_slots[wa], t_slots[wb]
            sa = ws[wa].rearrange("p (t c) -> p t c", c=C)
            sb = ws[wb].rearrange("p (t c) -> p t c", c=C)
            pa = ps_pool.tile([P, C], dt.float32, tag="psa")
            pb = ps_pool.tile([P, C], dt.float32, tag="psb")
            for t in range(max(ta, tb)):
                if t < ta:
                    nc.tensor.matmul(pa[:, :], lhsT=sa[:, t, 0:P],
                                     rhs=sa[:, t, :], start=(t == 0),
                                     stop=(t == ta - 1),
                                     skip_group_check=True)
                if t < tb:
                    nc.tensor.matmul(pb[:, :], lhsT=sb[:, t, 0:P],
                                     rhs=sb[:, t, :], start=(t == 0),
                                     stop=(t == tb - 1),
                                     skip_group_check=True)
            extract(pa, wa)
            extract(pb, wb)

        nc.sync.dma_start(out=o_d[:, :], in_=outs[:, :])

    nc.compile()
    return nc


def host_layout(pred, exp, group):
    """Size-sorted Gram-slab layout: per-core [P, F] fp8 + rank order."""
    from concourse import mybir
    ddt = mybir.dt.np(mybir.dt.float8e4) if USE_FP8 else ml_dtypes.bfloat16

    x = np.asarray(exp, dtype=np.float32)
    y = np.asarray(pred, dtype=np.float32)
    g = np.asarray(group).astype(np.int32)
    n = g.shape[0]

    sizes = np.bincount(g, minlength=G)
    # rank groups by size asc; rank r -> slot j = (r//64)//8, core (r//64)%8
    # (ascending: first processed pair is smallest -> shortest fill stall)
    grank = np.argsort(sizes, kind="stable")       # rank -> group id
    # per-slot depth: slot j spans ranks [512j, 512(j+1)); max is the last
    t_slots = tuple(
        max(int(np.ceil(int(sizes[grank[512 * j + 511]]) / P)), 1)
        for j in range(NW))
    offs = np.concatenate([[0], np.cumsum(np.array(t_slots))])  # slab offsets

    # destination of element i (of sorted-by-group stream):
    # group g at rank r: core = (r//64)%8, slot j, col m = r%64
    rank_of = np.empty(G, dtype=np.int64)
    rank_of[grank] = np.arange(G)
    order = np.argsort(g, kind="stable")
    gs = g[order].astype(np.int64)
    starts = np.zeros(G, dtype=np.int64)
    starts[1:] = np.cumsum(sizes)[:-1]
    pos = np.arange(n, dtype=np.int64) - starts[gs]   # position within group

    r = rank_of[gs]
    core = (r // H) % N_CORES
    slot = (r // H) // N_CORES
    col = r % H
    t = pos // P
    k = pos % P
    F = int(offs[-1]) * C
    # dst within [core][k][F]: (offs[slot] + t)*C + col_within_slab
    v = np.zeros((N_CORES, P, F), dtype=ddt)
    flat_f = (offs[slot] + t) * C
    for si, vv in enumerate((x, y)):
        v[core, k, flat_f + col + si * H] = vv.astype(ddt)[order]
    # ones columns
    for j in range(NW):
        for t_ in range(t_slots[j]):
            v[:, :, (int(offs[j]) + t_) * C + 2 * H] = ddt(1.0)
    return v, sizes.astype(np.float64), t_slots, grank


def _finish_host(S):
    n, sx, sy, sxy, sxx, syy = S
    n_safe = np.where(n > 0, n, 1.0)
    mx = sx / n_safe
    my = sy / n_safe
    cov = sxy / n_safe - mx * my
    var_x = sxx / n_safe - mx * mx
    var_y = syy / n_safe - my * my
    denom = np.sqrt(np.maximum(var_x * var_y, 0.0))
    corr = np.where(denom > 0, cov / np.where(denom > 0, denom, 1.0), 0.0)
    corr_pearson = np.sum(corr * n) / np.sum(n)
    return np.float32(-corr_pearson)


_NC_CACHE = {}


def _get_nc(t_slots):
    if t_slots not in _NC_CACHE:
        _NC_CACHE[t_slots] = build_nc(t_slots)
    return _NC_CACHE[t_slots]


def _install_ntff_hook():
    """bass_utils imports antenv.axon_hooks when tracing; this image lacks
    that submodule.  Provide it, wired to the axon ctypes NTFF hook, so a
    tracing harness does not crash.  Harmless when tracing is off."""
    import sys
    import types

    if "antenv.axon_hooks" in sys.modules:
        return
    try:
        import antenv

        mod = types.ModuleType("antenv.axon_hooks")
        hook = [None]
        mod.set_axon_ntff_profile_hook = lambda h: hook.__setitem__(0, h)
        mod.get_axon_ntff_profile_hook = lambda: hook[0]
        sys.modules["antenv.axon_hooks"] = mod
        antenv.axon_hooks = mod
        from trn_agent_boot.trn_boot import _ntff_profile_via_ctypes

        mod.set_axon_ntff_profile_hook(
            _ntff_profile_via_ctypes("/opt/axon/libaxon_pjrt.so"))
    except Exception:
        pass


def kernel(pred, exp, group, num_groups, _trace=False):
    _install_ntff_hook()
    from concourse.bass_utils import run_bass_kernel_spmd

    pred = np.asarray(pred)
    exp = np.asarray(exp)
    group = np.asarray(group)

    v, sizes, t_slots, grank = host_layout(pred, exp, group)
    nc = _get_nc(t_slots)

    idh = np.eye(H, dtype=ml_dtypes.bfloat16)
    ident = np.concatenate([idh, idh], axis=0)          # [128, 64]
    in_maps = [{"v": v[i], "ident": ident} for i in range(N_CORES)]

    res = run_bass_kernel_spmd(nc, in_maps, list(range(N_CORES)),
                               trace=_trace)

    # stats by rank: rank r = 64*(8*slot + core) + m
    Sr = np.zeros((5, G), dtype=np.float64)
    for i in range(N_CORES):
        o = res.results[i]["o"].astype(np.float64).reshape(P, NW, NO)
        r0 = H * i                                    # slot j block offset
        for j in range(NW):
            sl = slice(H * (N_CORES * j + i), H * (N_CORES * j + i) + H)
            Sr[3, sl] = o[0:H, j, 0]                  # sxx
            Sr[4, sl] = o[H:P, j, 0]                  # syy
            Sr[2, sl] = o[0:H, j, 1]                  # sxy
            Sr[0, sl] = o[0:H, j, 2]                  # sx
            Sr[1, sl] = o[H:P, j, 2]                  # sy
    S = np.zeros((6, G), dtype=np.float64)
    S[0] = sizes
    S[1][grank] = Sr[0]
    S[2][grank] = Sr[1]
    S[3][grank] = Sr[2]
    S[4][grank] = Sr[3]
    S[5][grank] = Sr[4]
    out = _finish_host(S)
    if _trace:
        return out, res
    return out
